# revision 1
# baseline (speedup 1.0000x reference)
"""Elman RNN (return_sequences=False) on 8 TRN2 NeuronCores.

Math (per core, batch shard of 32):
    proj[t]  = x[t] @ w + b          (big GEMMs, off critical path)
    s[0]     = tanh(proj[0])
    s[t]     = tanh(proj[t] + s[t-1] @ W)   t = 1..T-1
    out      = s[T-1]

Layout: everything lives transposed on-chip ([feature, batch]) so the
contraction dim is always the SBUF partition dim and no on-device
transposes are needed:
  - x is host-permuted per core to [D, T*Bs] (d-major, contiguous per
    partition -> full-bandwidth DMA).
  - proj^T for 16 timesteps at a time is written straight into one PSUM
    bank by a single N=512 matmul (lhsT=w, rhs=x chunk).
  - each recurrence step accumulates Ws^T @ s into its 32-col PSUM slice
    (start=False), then ACT does tanh(psum + bias) -> next state tile.
"""

import numpy as np

import concourse.bass as bass  # noqa: F401  (registers engine namespaces)
import concourse.bacc as bacc
import concourse.tile as tile
from concourse import mybir
from concourse.bass_utils import run_bass_kernel_spmd

B, T, D, H = 256, 1024, 128, 128
NCORES = 8
BS = B // NCORES  # batch per core
F32 = mybir.dt.float32

BLK_T = 16     # timesteps per PSUM bank (16*32 cols = 512 f32 = 1 bank)
CHUNK_T = 128  # timesteps per x DMA chunk (128*32 cols * 4B = 16KB/partition)


def build(T_=T):
    nc = bacc.Bacc("TRN2", target_bir_lowering=False, debug=False,
                   num_devices=NCORES)
    x_d = nc.dram_tensor("x", [D, T_ * BS], F32, kind="ExternalInput")
    w_d = nc.dram_tensor("w", [D, H], F32, kind="ExternalInput")
    sw_d = nc.dram_tensor("sw", [H, H], F32, kind="ExternalInput")
    b_d = nc.dram_tensor("b", [H, 1], F32, kind="ExternalInput")
    out_d = nc.dram_tensor("out", [H, BS], F32, kind="ExternalOutput")

    tanh = mybir.ActivationFunctionType.Tanh

    with tile.TileContext(nc) as tc:
        with tc.tile_pool(name="const", bufs=1) as cpool, \
             tc.tile_pool(name="xp", bufs=2) as xpool, \
             tc.tile_pool(name="pp", bufs=8, space="PSUM") as ppool, \
             tc.tile_pool(name="sp", bufs=3) as spool:
            w_sb = cpool.tile([D, H], F32, tag="w")
            nc.sync.dma_start(w_sb[:], w_d.ap())
            sw_sb = cpool.tile([H, H], F32, tag="sw")
            nc.sync.dma_start(sw_sb[:], sw_d.ap())
            b_sb = cpool.tile([H, 1], F32, tag="b")
            nc.sync.dma_start(b_sb[:], b_d.ap())

            state = None
            xt = None
            for blk in range(T_ // BLK_T):
                t0 = blk * BLK_T
                if t0 % CHUNK_T == 0:
                    xt = xpool.tile([D, CHUNK_T * BS], F32, tag="x")
                    c0 = t0 * BS
                    nc.sync.dma_start(xt[:], x_d.ap()[:, c0:c0 + CHUNK_T * BS])
                off = (t0 % CHUNK_T) * BS
                ps = ppool.tile([H, BLK_T * BS], F32, tag="ps")
                # proj^T for 16 steps -> whole bank (start=True zeroes it)
                nc.tensor.matmul(ps[:], w_sb[:], xt[:, off:off + BLK_T * BS],
                                 start=True, stop=False, skip_group_check=True)
                for k in range(BLK_T):
                    t = t0 + k
                    sl = ps[:, k * BS:(k + 1) * BS]
                    if t > 0:
                        nc.tensor.matmul(sl, sw_sb[:], state[:],
                                         start=False, stop=(k == BLK_T - 1),
                                         skip_group_check=True)
                    snew = spool.tile([H, BS], F32, tag="st")
                    nc.scalar.activation(snew[:], sl, tanh, bias=b_sb[:, 0:1])
                    state = snew
            nc.sync.dma_start(out_d.ap(), state[:])
    nc.compile()
    return nc


def shard_inputs(x, w, state_weight, b):
    """Full inputs -> per-core in_maps (batch-sharded x, replicated weights)."""
    w = np.ascontiguousarray(w, dtype=np.float32)
    sw = np.ascontiguousarray(state_weight, dtype=np.float32)
    bb = np.ascontiguousarray(b, dtype=np.float32).reshape(H, 1)
    in_maps = []
    for i in range(NCORES):
        xs = x[i * BS:(i + 1) * BS]                      # [Bs, T, D]
        xs = np.ascontiguousarray(xs.transpose(2, 1, 0), dtype=np.float32)
        in_maps.append({"x": xs.reshape(D, -1), "w": w, "sw": sw, "b": bb})
    return in_maps


_NC = None


def kernel(x, w, state_weight, b, **run_kwargs):
    global _NC
    if _NC is None:
        _NC = build()
    in_maps = shard_inputs(x, w, state_weight, b)
    res = run_bass_kernel_spmd(_NC, in_maps, core_ids=list(range(NCORES)),
                               **run_kwargs)
    out = np.concatenate([r["out"].T for r in res.results], axis=0)  # [B, H]
    if run_kwargs:
        return out, res
    return out


if __name__ == "__main__":
    rng = np.random.default_rng(0)
    x = rng.standard_normal((B, T, D), dtype=np.float32)
    w = (rng.standard_normal((D, H), dtype=np.float32) * 0.05)
    sw = (rng.standard_normal((H, H), dtype=np.float32) * 0.05)
    b = (rng.standard_normal((H,), dtype=np.float32) * 0.05)
    out = kernel(x, w, sw, b)
    print(out.shape, out.dtype, np.abs(out).mean())


# revision 2
# speedup vs baseline: 1.8244x; 1.8244x over previous
"""Elman RNN (return_sequences=False) on 8 TRN2 NeuronCores.

Math (per core, batch shard of 32):
    proj[t]  = x[t] @ w + b          (big GEMMs, off critical path)
    s[0]     = tanh(proj[0])
    s[t]     = tanh(proj[t] + s[t-1] @ W)   t = 1..T-1
    out      = s[T-1]

Layout: everything lives transposed on-chip ([feature, batch]) so the
contraction dim is always the SBUF partition dim and no on-device
transposes are needed:
  - x is host-permuted per core to [D, T*Bs] (d-major, contiguous per
    partition -> full-bandwidth DMA).
  - proj^T for 16 timesteps at a time is written straight into one PSUM
    bank by a single N=512 matmul (lhsT=w, rhs=x chunk).
  - each recurrence step accumulates Ws^T @ s into its 32-col PSUM slice
    (start=False), then ACT does tanh(psum + bias) -> next state tile.

Matmul inputs are bf16: fp32 matmuls lower to 2 half-speed PE passes +
slow (no-FWL) weight loads, tripling the serial-chain latency. PSUM
accumulation and tanh stay fp32; end-to-end max rel err ~7e-3.
"""

import numpy as np
import ml_dtypes

import concourse.bass as bass  # noqa: F401  (registers engine namespaces)
import concourse.bacc as bacc
import concourse.tile as tile
from concourse import mybir
from concourse.bass_utils import run_bass_kernel_spmd

B, T, D, H = 256, 1024, 128, 128
NCORES = 8
BS = B // NCORES  # batch per core
F32 = mybir.dt.float32
BF16 = mybir.dt.bfloat16

BLK_T = 16     # timesteps per PSUM bank (16*32 cols = 512 f32 = 1 bank)
CHUNK_T = 128  # timesteps per x DMA chunk

PROJ_BF16 = True   # x/w (input projection GEMM) in bf16
REC_BF16 = True    # state_weight/state (recurrence matmul) in bf16


def build(T_=T, proj_bf16=PROJ_BF16, rec_bf16=REC_BF16):
    xdt = BF16 if proj_bf16 else F32
    sdt = BF16 if rec_bf16 else F32
    nc = bacc.Bacc("TRN2", target_bir_lowering=False, debug=False,
                   num_devices=NCORES)
    x_d = nc.dram_tensor("x", [D, T_ * BS], xdt, kind="ExternalInput")
    w_d = nc.dram_tensor("w", [D, H], xdt, kind="ExternalInput")
    sw_d = nc.dram_tensor("sw", [H, H], sdt, kind="ExternalInput")
    b_d = nc.dram_tensor("b", [H, 1], F32, kind="ExternalInput")
    out_d = nc.dram_tensor("out", [H, BS], F32, kind="ExternalOutput")

    tanh = mybir.ActivationFunctionType.Tanh

    with tile.TileContext(nc) as tc:
        with tc.tile_pool(name="const", bufs=1) as cpool, \
             tc.tile_pool(name="xp", bufs=2) as xpool, \
             tc.tile_pool(name="pp", bufs=8, space="PSUM") as ppool, \
             tc.tile_pool(name="sp", bufs=3) as spool:
            w_sb = cpool.tile([D, H], xdt, tag="w")
            nc.sync.dma_start(w_sb[:], w_d.ap())
            sw_sb = cpool.tile([H, H], sdt, tag="sw")
            nc.sync.dma_start(sw_sb[:], sw_d.ap())
            b_sb = cpool.tile([H, 1], F32, tag="b")
            nc.sync.dma_start(b_sb[:], b_d.ap())

            state = None
            xt = None
            nblk = T_ // BLK_T
            for blk in range(nblk):
                t0 = blk * BLK_T
                if t0 % CHUNK_T == 0:
                    xt = xpool.tile([D, CHUNK_T * BS], xdt, tag="x")
                    c0 = t0 * BS
                    nc.sync.dma_start(xt[:], x_d.ap()[:, c0:c0 + CHUNK_T * BS])
                off = (t0 % CHUNK_T) * BS
                ps = ppool.tile([H, BLK_T * BS], F32, tag="ps")
                # proj^T for 16 steps -> whole bank (start=True zeroes it)
                nc.tensor.matmul(ps[:], w_sb[:], xt[:, off:off + BLK_T * BS],
                                 start=True, stop=False, skip_group_check=True)
                for k in range(BLK_T):
                    t = t0 + k
                    sl = ps[:, k * BS:(k + 1) * BS]
                    if t > 0:
                        nc.tensor.matmul(sl, sw_sb[:], state[:],
                                         start=False, stop=(k == BLK_T - 1),
                                         skip_group_check=True)
                    last = t == T_ - 1
                    snew = spool.tile([H, BS], F32 if last else sdt,
                                      tag="stf" if last else "st")
                    nc.scalar.activation(snew[:], sl, tanh, bias=b_sb[:, 0:1])
                    state = snew
            nc.sync.dma_start(out_d.ap(), state[:])
    nc.compile()
    return nc


def shard_inputs(x, w, state_weight, b, proj_bf16=PROJ_BF16, rec_bf16=REC_BF16):
    """Full inputs -> per-core in_maps (batch-sharded x, replicated weights)."""
    xnp = ml_dtypes.bfloat16 if proj_bf16 else np.float32
    snp = ml_dtypes.bfloat16 if rec_bf16 else np.float32
    w = np.ascontiguousarray(w.astype(xnp))
    sw = np.ascontiguousarray(state_weight.astype(snp))
    bb = np.ascontiguousarray(b, dtype=np.float32).reshape(H, 1)
    in_maps = []
    for i in range(NCORES):
        xs = x[i * BS:(i + 1) * BS]                      # [Bs, T, D]
        xs = np.ascontiguousarray(xs.transpose(2, 1, 0).astype(xnp))
        in_maps.append({"x": xs.reshape(D, -1), "w": w, "sw": sw, "b": bb})
    return in_maps


_NC = None


def kernel(x, w, state_weight, b, **run_kwargs):
    global _NC
    if _NC is None:
        _NC = build()
    in_maps = shard_inputs(x, w, state_weight, b)
    res = run_bass_kernel_spmd(_NC, in_maps, core_ids=list(range(NCORES)),
                               **run_kwargs)
    out = np.concatenate([r["out"].T for r in res.results], axis=0)  # [B, H]
    if run_kwargs:
        return out, res
    return out


if __name__ == "__main__":
    rng = np.random.default_rng(0)
    x = rng.standard_normal((B, T, D), dtype=np.float32)
    w = (rng.standard_normal((D, H), dtype=np.float32) * 0.05)
    sw = (rng.standard_normal((H, H), dtype=np.float32) * 0.05)
    b = (rng.standard_normal((H,), dtype=np.float32) * 0.05)
    out = kernel(x, w, sw, b)
    print(out.shape, out.dtype, np.abs(out).mean())
